# revision 17
# baseline (speedup 1.0000x reference)
"""DAS (delay-and-sum) beamforming kernel for Trainium2, 8 NeuronCores.

out[b, z, x, k] = sum_nc( (1-w)*rfs[b,k,nc,i0] + w*rfs[b,k,nc,i0+1] ),
idx = samples_idx[ids[b], nc, z, x], i0 = floor(idx), w = idx - i0.

Strategy (pixel sharding): 65536 pixels / 8 cores = 8192 per core; rfs
replicated. Per core, 16 passes over the 128 (b,nc) pairs (8 per pass).

  - SBUF table per pass (host pre-interleaved):
      partition 16g+k   = rfs[b,k,nc,:]            (v0 rows)
      partition 16g+8+k = rfs[b,k,nc,1:] ++ [0]    (v1 rows, shifted)
    One GPSIMD ap_gather with the per-group shared pixel index i0 fetches
    v0=S[i0] and v1=S[i0+1] for all 8 k at once.
  - floor/frac are computed ON HOST: the kernel receives the wrapped
    int16 index tile directly plus compact bf16 frac rows [NPASS*8, SH].
    SBUF DMA dst APs map only dim0 to partitions, so the 8->128 lane
    expansion is 16 strided DMAs per pass (dst partitions j::16), issued
    alternately from the SP and ACT sequencers.
  - out = sum v0 + sum w*(v1 - v0): PE accumulates two matmuls per
    128-pixel block: raw G (fp32) against sel0 (+1 on v0 lanes) and
    GW = G*w (bf16, separate tile to decouple engine pipelines) against
    selpm (-1 on v0, +1 on v1 lanes); psum[pixel, k] accumulated over
    the 8 passes of each b.
"""
import numpy as np
import ml_dtypes

import concourse.bacc as bacc
import concourse.tile as tile
import concourse.mybir as mybir
from concourse.bass_utils import run_bass_kernel_spmd

dt = mybir.dt

B, K, NC, NS = 2, 8, 64, 2048
NZ, NX = 256, 256
NPIX = NZ * NX
NCORES = 8
SH = NPIX // NCORES          # pixels per core = 8192
NPASS = (B * NC) // 8        # 16 passes, 8 (b,nc) groups per pass
BLK = 128                    # pixels per matmul weight-load
NBLK = SH // BLK             # 64
CW = SH // 16                # wrapped idx columns per pass = 512

_CACHE = {}


def _build_program():
    nc = bacc.Bacc(
        "TRN2",
        target_bir_lowering=False,
        debug=False,
        dynamic_dma_scratch_size=16384,
    )
    tab_d = nc.dram_tensor("tab", [NPASS, 128, NS], dt.float32, kind="ExternalInput")
    i16_d = nc.dram_tensor("i16w", [NPASS, 128, CW], dt.int16,
                           kind="ExternalInput")
    wrep_d = nc.dram_tensor("wrep", [NPASS * 32, SH], dt.bfloat16,
                            kind="ExternalInput")
    sel_d = nc.dram_tensor("sel", [128, K], dt.float32, kind="ExternalInput")
    selb_d = nc.dram_tensor("selb", [128, K], dt.bfloat16, kind="ExternalInput")
    out_d = nc.dram_tensor("out", [B, 128, NBLK * K], dt.float32,
                           kind="ExternalOutput")

    with tile.TileContext(nc) as tc:
        from contextlib import ExitStack
        with ExitStack() as ctx:
            tp = ctx.enter_context(tc.tile_pool(name="tabs", bufs=2))
            gp = ctx.enter_context(tc.tile_pool(name="gath", bufs=2))
            wp = ctx.enter_context(tc.tile_pool(name="gw", bufs=2))
            fp = ctx.enter_context(tc.tile_pool(name="frac", bufs=2))
            ip = ctx.enter_context(tc.tile_pool(name="idxs", bufs=1))
            sp = ctx.enter_context(tc.tile_pool(name="small", bufs=1))
            pp = ctx.enter_context(tc.tile_pool(name="ps", bufs=1, space="PSUM"))

            # critical-path first: the p=0 table load gates the first gather,
            # so it goes ahead of everything else on the SP DMA queue
            T_tiles = {}
            T_tiles[0] = tp.tile([128, NS], dt.float32, tag="T", name="T0")
            nc.sync.dma_start(T_tiles[0][:, :], tab_d[0, :, :])

            # per-pass index tiles, 3 prefetched up front, rest in-loop
            i16 = {}
            for p in range(min(3, NPASS)):
                i16[p] = ip.tile([128, CW], dt.int16, tag=f"i16_{p}",
                                 name=f"i16_{p}")
                nc.sync.dma_start(i16[p][:, :], i16_d[p, :, :])

            sel_t = sp.tile([128, K], dt.float32, name="sel_t")
            nc.sync.dma_start(sel_t[:, :], sel_d[:, :])
            selb_t = sp.tile([128, K], dt.bfloat16, name="selb_t")
            nc.scalar.dma_start(selb_t[:, :], selb_d[:, :])

            psums = []
            for b in range(B):
                pst = pp.tile([128, NBLK * K], dt.float32, tag=f"ps{b}",
                              name=f"ps{b}")
                psums.append(pst)

            for p in range(NPASS):
                b = p // 8

                # --- (v0, v1) table rows for this pass (host-interleaved) ---
                if p not in T_tiles:
                    T_tiles[p] = tp.tile([128, NS], dt.float32, tag="T",
                                         name=f"T{p}")
                    nc.sync.dma_start(T_tiles[p][:, :], tab_d[p, :, :])
                T = T_tiles[p]
                if p + 3 < NPASS:
                    q = p + 3
                    i16[q] = ip.tile([128, CW], dt.int16, tag=f"i16_{q}",
                                     name=f"i16_{q}")
                    nc.sync.dma_start(i16[q][:, :], i16_d[q, :, :])

                # --- all-lane w tile: host replicates each group row 4x, so
                # 4 strided DMAs (dst partitions j0::4 <- 32 linear rows)
                # complete the 16x lane expansion: F[4m+j0] = w[m//4] and
                # floor((4m+j0)/16) == m//4 for j0 < 4.
                F = fp.tile([128, SH], dt.bfloat16, tag="F")
                for j0 in range(4):
                    eng = nc.scalar if j0 % 2 else nc.sync
                    eng.dma_start(F[j0::4, :], wrep_d[p * 32:(p + 1) * 32, :])

                # --- the gather: G[16g+j, q] = T[16g+j, i0[g,q]] ---
                G = gp.tile([128, SH], dt.float32, tag="G")
                nc.gpsimd.ap_gather(
                    G[:, :].rearrange("p (n i) -> p n i", i=1),
                    T[:, :].rearrange("p (n i) -> p n i", i=1),
                    i16[p][:, :],
                    channels=128,
                    num_elems=NS,
                    d=1,
                    num_idxs=SH,
                )

                # --- weighted copy on DVE (separate tile: no PE-order dep).
                # Kept off GPSIMD entirely: a Pool tensor op would force a
                # Q7 ucode-library reload away from the ap_gather library.
                GW = wp.tile([128, SH], dt.bfloat16, tag="GW")
                nc.vector.tensor_mul(GW[:, :], G[:, :], F[:, :])

                # --- PE: raw G vs sel0, then GW vs selpm, accumulated ---
                # NB: start=True resets the WHOLE psum bank, so only the
                # first matmul of each bank's group sets it; only the very
                # last sets stop.
                for blk in range(NBLK):
                    nc.tensor.matmul(
                        psums[b][:, blk * K:(blk + 1) * K],
                        G[:, blk * BLK:(blk + 1) * BLK],
                        sel_t[:, :],
                        start=(p % 8 == 0 and blk == 0),
                        stop=False,
                        skip_group_check=True,
                    )
                for blk in range(NBLK):
                    nc.tensor.matmul(
                        psums[b][:, blk * K:(blk + 1) * K],
                        GW[:, blk * BLK:(blk + 1) * BLK],
                        selb_t[:, :],
                        start=False,
                        stop=(p % 8 == 7 and blk == NBLK - 1),
                        skip_group_check=True,
                    )

                if p % 8 == 7:
                    cp = sp.tile([128, NBLK * K], dt.float32, tag=f"cp{b}",
                                 name=f"cp{b}")
                    nc.scalar.copy(cp[:, :], psums[b][:, :])
                    nc.sync.dma_start(out_d[b, :, :], cp[:, :])

    nc.compile()
    return nc


def _host_prep(rfs, ids, samples_idx):
    rfs = np.asarray(rfs, dtype=np.float32)
    ids = np.asarray(ids).astype(np.int64)
    samples_idx = np.asarray(samples_idx, dtype=np.float32)

    # table rows: tab[p, 16g+k] = rfs[b, k, nc, :]; tab[p, 16g+8+k] = shifted
    s_rows = rfs.transpose(0, 2, 1, 3)                           # b, nc, k, s
    sh_rows = np.zeros_like(s_rows)
    sh_rows[..., : NS - 1] = s_rows[..., 1:]
    both = np.stack([s_rows, sh_rows], axis=2)                   # b, nc, tap, k, s
    tab = np.ascontiguousarray(both.reshape(NPASS, 128, NS))

    idx = samples_idx[ids].reshape(B, NC, NPIX)  # [2, 64, 65536]
    i0 = np.floor(idx)
    i16full = i0.astype(np.int16)
    wfull = (idx - i0).astype(ml_dtypes.bfloat16)

    # slot = 16g + 8t + k;  sel: +1 on t==0 lanes (raw G term)
    # selb: -1 on t==0, +1 on t==1 lanes (w*(v1-v0) term)
    sel = np.zeros((128, K), dtype=np.float32)
    selb = np.zeros((128, K), dtype=np.float32)
    slots = np.arange(128)
    t_of = (slots % 16) // 8
    k_of = slots % 8
    sel[slots, k_of] = (t_of == 0).astype(np.float32)
    selb[slots, k_of] = np.where(t_of == 0, -1.0, 1.0)
    selb = selb.astype(ml_dtypes.bfloat16)

    in_maps = []
    for c in range(NCORES):
        sl16 = i16full[:, :, c * SH:(c + 1) * SH]    # [B, NC, SH]
        # wrapped: [NPASS, 128, CW]: partition 16g+m, free col
        t = sl16.reshape(B, 8, 8, CW, 16)            # b, ncg, g, c, m
        t = t.transpose(0, 1, 2, 4, 3)               # b, ncg, g, m, c
        t = t.reshape(NPASS, 8, 16, CW).reshape(NPASS, 128, CW)
        i16w = np.ascontiguousarray(t)
        # frac rows replicated 4x: row p*32 + 4g + u = w[b, nc, core slice]
        wrep = np.ascontiguousarray(np.repeat(
            wfull[:, :, c * SH:(c + 1) * SH].reshape(NPASS * 8, SH), 4, axis=0
        ))
        in_maps.append(dict(tab=tab, i16w=i16w, wrep=wrep, sel=sel, selb=selb))
    return in_maps


def kernel(rfs, ids, samples_idx):
    if "nc" not in _CACHE:
        _CACHE["nc"] = _build_program()
    nc = _CACHE["nc"]

    in_maps = _host_prep(rfs, ids, samples_idx)
    res = run_bass_kernel_spmd(nc, in_maps, core_ids=list(range(NCORES)))

    out = np.empty((B, NPIX, K), dtype=np.float32)
    for c in range(NCORES):
        o = res.results[c]["out"]                     # [B, 128, NBLK*K]
        o = o.reshape(B, 128, NBLK, K).transpose(0, 2, 1, 3)  # b, blk, qlo, k
        out[:, c * SH:(c + 1) * SH, :] = o.reshape(B, SH, K)
    return out.reshape(B, NZ, NX, K)


# revision 20
# speedup vs baseline: 1.1768x; 1.1768x over previous
"""DAS (delay-and-sum) beamforming kernel for Trainium2, 8 NeuronCores.

out[b, z, x, k] = sum_nc( (1-w)*rfs[b,k,nc,i0] + w*rfs[b,k,nc,i0+1] ),
idx = samples_idx[ids[b], nc, z, x], i0 = floor(idx), w = idx - i0.

Strategy (pixel sharding): 65536 pixels / 8 cores = 8192 per core; rfs
replicated. Per core, 16 passes over the 128 (b,nc) pairs (8 per pass).

  - SBUF table per pass (host pre-interleaved):
      partition 16g+k   = rfs[b,k,nc,:]            (v0 rows)
      partition 16g+8+k = rfs[b,k,nc,1:] ++ [0]    (v1 rows, shifted)
    One GPSIMD ap_gather with the per-group shared pixel index i0 fetches
    v0=S[i0] and v1=S[i0+1] for all 8 k at once.
  - floor/frac are computed ON HOST: the kernel receives the wrapped
    int16 index tile directly plus compact bf16 frac rows [NPASS*8, SH].
    SBUF DMA dst APs map only dim0 to partitions, so the 8->128 lane
    expansion is 16 strided DMAs per pass (dst partitions j::16), issued
    alternately from the SP and ACT sequencers.
  - out = sum v0 + sum w*(v1 - v0): PE accumulates two matmuls per
    128-pixel block: raw G (fp32) against sel0 (+1 on v0 lanes) and
    GW = G*w (bf16, separate tile to decouple engine pipelines) against
    selpm (-1 on v0, +1 on v1 lanes); psum[pixel, k] accumulated over
    the 8 passes of each b.
"""
import numpy as np
import ml_dtypes

import concourse.bacc as bacc
import concourse.tile as tile
import concourse.mybir as mybir
from concourse.bass_utils import run_bass_kernel_spmd

dt = mybir.dt

B, K, NC, NS = 2, 8, 64, 2048
NZ, NX = 256, 256
NPIX = NZ * NX
NCORES = 8
SH = NPIX // NCORES          # pixels per core = 8192
NPASS = (B * NC) // 8        # 16 passes, 8 (b,nc) groups per pass
BLK = 128                    # pixels per matmul weight-load
NBLK = SH // BLK             # 64
CW = SH // 16                # wrapped idx columns per pass = 512

_CACHE = {}


def _build_program():
    nc = bacc.Bacc(
        "TRN2",
        target_bir_lowering=False,
        debug=False,
        dynamic_dma_scratch_size=16384,
    )
    tab_d = nc.dram_tensor("tab", [NPASS, 128, NS], dt.float32, kind="ExternalInput")
    i16_d = nc.dram_tensor("i16w", [NPASS, 128, CW], dt.int16,
                           kind="ExternalInput")
    wrep_d = nc.dram_tensor("wrep", [NPASS * 8, SH], dt.bfloat16,
                            kind="ExternalInput")
    sel_d = nc.dram_tensor("sel", [128, K], dt.float32, kind="ExternalInput")
    selb_d = nc.dram_tensor("selb", [128, K], dt.bfloat16, kind="ExternalInput")
    out_d = nc.dram_tensor("out", [B, 128, NBLK * K], dt.float32,
                           kind="ExternalOutput")

    with tile.TileContext(nc) as tc:
        from contextlib import ExitStack
        with ExitStack() as ctx:
            tp = ctx.enter_context(tc.tile_pool(name="tabs", bufs=2))
            gp = ctx.enter_context(tc.tile_pool(name="gath", bufs=2))
            wp = ctx.enter_context(tc.tile_pool(name="gw", bufs=2))
            fp = ctx.enter_context(tc.tile_pool(name="frac", bufs=2))
            ip = ctx.enter_context(tc.tile_pool(name="idxs", bufs=1))
            sp = ctx.enter_context(tc.tile_pool(name="small", bufs=1))
            pp = ctx.enter_context(tc.tile_pool(name="ps", bufs=1, space="PSUM"))

            # critical-path first: the p=0 table load gates the first gather,
            # so it goes ahead of everything else on the SP DMA queue
            T_tiles = {}
            T_tiles[0] = tp.tile([128, NS], dt.float32, tag="T", name="T0")
            nc.sync.dma_start(T_tiles[0][:, :], tab_d[0, :, :])

            # per-pass index tiles, 3 prefetched up front, rest in-loop
            i16 = {}
            for p in range(min(3, NPASS)):
                i16[p] = ip.tile([128, CW], dt.int16, tag=f"i16_{p}",
                                 name=f"i16_{p}")
                nc.sync.dma_start(i16[p][:, :], i16_d[p, :, :])

            sel_t = sp.tile([128, K], dt.float32, name="sel_t")
            nc.sync.dma_start(sel_t[:, :], sel_d[:, :])
            selb_t = sp.tile([128, K], dt.bfloat16, name="selb_t")
            nc.scalar.dma_start(selb_t[:, :], selb_d[:, :])

            psums = []
            for b in range(B):
                pst = pp.tile([128, NBLK * K], dt.float32, tag=f"ps{b}",
                              name=f"ps{b}")
                psums.append(pst)

            for p in range(NPASS):
                b = p // 8

                # --- (v0, v1) table rows for this pass (host-interleaved) ---
                if p not in T_tiles:
                    T_tiles[p] = tp.tile([128, NS], dt.float32, tag="T",
                                         name=f"T{p}")
                    nc.sync.dma_start(T_tiles[p][:, :], tab_d[p, :, :])
                T = T_tiles[p]
                if p + 3 < NPASS:
                    q = p + 3
                    i16[q] = ip.tile([128, CW], dt.int16, tag=f"i16_{q}",
                                     name=f"i16_{q}")
                    nc.sync.dma_start(i16[q][:, :], i16_d[q, :, :])

                # --- all-lane w tile: 16 strided DMAs (dst partitions j::16
                # <- the 8 compact rows) do the lane expansion. Input staging
                # is charged per exec (~18us/MB/core), so compact beats
                # host-replicated; the extra HWDGE issues hide under the
                # 200us+ gather.
                F = fp.tile([128, SH], dt.bfloat16, tag="F")
                for j in range(16):
                    eng = nc.scalar if j % 2 else nc.sync
                    eng.dma_start(F[j::16, :], wrep_d[p * 8:(p + 1) * 8, :])

                # --- the gather: G[16g+j, q] = T[16g+j, i0[g,q]] ---
                G = gp.tile([128, SH], dt.float32, tag="G")
                nc.gpsimd.ap_gather(
                    G[:, :].rearrange("p (n i) -> p n i", i=1),
                    T[:, :].rearrange("p (n i) -> p n i", i=1),
                    i16[p][:, :],
                    channels=128,
                    num_elems=NS,
                    d=1,
                    num_idxs=SH,
                )

                # --- weighted copy on DVE (separate tile: no PE-order dep).
                # Kept off GPSIMD entirely: a Pool tensor op would force a
                # Q7 ucode-library reload away from the ap_gather library.
                GW = wp.tile([128, SH], dt.bfloat16, tag="GW")
                nc.vector.tensor_mul(GW[:, :], G[:, :], F[:, :])

                # --- PE: raw G vs sel0, then GW vs selpm, accumulated ---
                # NB: start=True resets the WHOLE psum bank, so only the
                # first matmul of each bank's group sets it; only the very
                # last sets stop.
                for blk in range(NBLK):
                    nc.tensor.matmul(
                        psums[b][:, blk * K:(blk + 1) * K],
                        G[:, blk * BLK:(blk + 1) * BLK],
                        sel_t[:, :],
                        start=(p % 8 == 0 and blk == 0),
                        stop=False,
                        skip_group_check=True,
                    )
                for blk in range(NBLK):
                    nc.tensor.matmul(
                        psums[b][:, blk * K:(blk + 1) * K],
                        GW[:, blk * BLK:(blk + 1) * BLK],
                        selb_t[:, :],
                        start=False,
                        stop=(p % 8 == 7 and blk == NBLK - 1),
                        skip_group_check=True,
                    )

                if p % 8 == 7:
                    cp = sp.tile([128, NBLK * K], dt.float32, tag=f"cp{b}",
                                 name=f"cp{b}")
                    nc.scalar.copy(cp[:, :], psums[b][:, :])
                    nc.sync.dma_start(out_d[b, :, :], cp[:, :])

    nc.compile()
    return nc


def _host_prep(rfs, ids, samples_idx):
    rfs = np.asarray(rfs, dtype=np.float32)
    ids = np.asarray(ids).astype(np.int64)
    samples_idx = np.asarray(samples_idx, dtype=np.float32)

    # table rows: tab[p, 16g+k] = rfs[b, k, nc, :]; tab[p, 16g+8+k] = shifted
    s_rows = rfs.transpose(0, 2, 1, 3)                           # b, nc, k, s
    sh_rows = np.zeros_like(s_rows)
    sh_rows[..., : NS - 1] = s_rows[..., 1:]
    both = np.stack([s_rows, sh_rows], axis=2)                   # b, nc, tap, k, s
    tab = np.ascontiguousarray(both.reshape(NPASS, 128, NS))

    idx = samples_idx[ids].reshape(B, NC, NPIX)  # [2, 64, 65536]
    i0 = np.floor(idx)
    i16full = i0.astype(np.int16)
    wfull = (idx - i0).astype(ml_dtypes.bfloat16)

    # slot = 16g + 8t + k;  sel: +1 on t==0 lanes (raw G term)
    # selb: -1 on t==0, +1 on t==1 lanes (w*(v1-v0) term)
    sel = np.zeros((128, K), dtype=np.float32)
    selb = np.zeros((128, K), dtype=np.float32)
    slots = np.arange(128)
    t_of = (slots % 16) // 8
    k_of = slots % 8
    sel[slots, k_of] = (t_of == 0).astype(np.float32)
    selb[slots, k_of] = np.where(t_of == 0, -1.0, 1.0)
    selb = selb.astype(ml_dtypes.bfloat16)

    in_maps = []
    for c in range(NCORES):
        sl16 = i16full[:, :, c * SH:(c + 1) * SH]    # [B, NC, SH]
        # wrapped: [NPASS, 128, CW]: partition 16g+m, free col
        t = sl16.reshape(B, 8, 8, CW, 16)            # b, ncg, g, c, m
        t = t.transpose(0, 1, 2, 4, 3)               # b, ncg, g, m, c
        t = t.reshape(NPASS, 8, 16, CW).reshape(NPASS, 128, CW)
        i16w = np.ascontiguousarray(t)
        # compact frac rows: row p*8+g = w[b, nc, core slice], pixel order
        wrep = np.ascontiguousarray(
            wfull[:, :, c * SH:(c + 1) * SH].reshape(NPASS * 8, SH)
        )
        in_maps.append(dict(tab=tab, i16w=i16w, wrep=wrep, sel=sel, selb=selb))
    return in_maps


def kernel(rfs, ids, samples_idx):
    if "nc" not in _CACHE:
        _CACHE["nc"] = _build_program()
    nc = _CACHE["nc"]

    in_maps = _host_prep(rfs, ids, samples_idx)
    res = run_bass_kernel_spmd(nc, in_maps, core_ids=list(range(NCORES)))

    out = np.empty((B, NPIX, K), dtype=np.float32)
    for c in range(NCORES):
        o = res.results[c]["out"]                     # [B, 128, NBLK*K]
        o = o.reshape(B, 128, NBLK, K).transpose(0, 2, 1, 3)  # b, blk, qlo, k
        out[:, c * SH:(c + 1) * SH, :] = o.reshape(B, SH, K)
    return out.reshape(B, NZ, NX, K)


# revision 25
# speedup vs baseline: 1.1827x; 1.0049x over previous
"""DAS (delay-and-sum) beamforming kernel for Trainium2, 8 NeuronCores.

out[b, z, x, k] = sum_nc( (1-w)*rfs[b,k,nc,i0] + w*rfs[b,k,nc,i0+1] ),
idx = samples_idx[ids[b], nc, z, x], i0 = floor(idx), w = idx - i0.

Strategy (pixel sharding): 65536 pixels / 8 cores = 8192 per core; rfs
replicated. Per core, 16 passes over the 128 (b,nc) pairs (8 per pass).

  - SBUF table per pass (host pre-interleaved):
      partition 16g+k   = rfs[b,k,nc,:]            (v0 rows)
      partition 16g+8+k = rfs[b,k,nc,1:] ++ [0]    (v1 rows, shifted)
    One GPSIMD ap_gather with the per-group shared pixel index i0 fetches
    v0=S[i0] and v1=S[i0+1] for all 8 k at once.
  - floor/frac are computed ON HOST: the kernel receives the wrapped
    int16 index tile directly plus compact bf16 frac rows [NPASS*8, SH].
    SBUF DMA dst APs map only dim0 to partitions, so the 8->128 lane
    expansion is 16 strided DMAs per pass (dst partitions j::16), issued
    alternately from the SP and ACT sequencers.
  - out = sum v0 + sum w*(v1 - v0): PE accumulates two matmuls per
    128-pixel block: raw G (fp32) against sel0 (+1 on v0 lanes) and
    GW = G*w (bf16, separate tile to decouple engine pipelines) against
    selpm (-1 on v0, +1 on v1 lanes); psum[pixel, k] accumulated over
    the 8 passes of each b.
"""
import numpy as np
import ml_dtypes

import concourse.bacc as bacc
import concourse.tile as tile
import concourse.mybir as mybir
from concourse.bass_utils import run_bass_kernel_spmd

dt = mybir.dt

B, K, NC, NS = 2, 8, 64, 2048
NZ, NX = 256, 256
NPIX = NZ * NX
NCORES = 8
SH = NPIX // NCORES          # pixels per core = 8192
NPASS = (B * NC) // 8        # 16 passes, 8 (b,nc) groups per pass
BLK = 128                    # pixels per matmul weight-load
NBLK = SH // BLK             # 64
CW = SH // 16                # wrapped idx columns per pass = 512

_CACHE = {}


def _build_program():
    nc = bacc.Bacc(
        "TRN2",
        target_bir_lowering=False,
        debug=False,
        dynamic_dma_scratch_size=16384,
    )
    tab_d = nc.dram_tensor("tab", [NPASS, 128, NS], dt.float32, kind="ExternalInput")
    i16_d = nc.dram_tensor("i16w", [NPASS, 128, CW], dt.int16,
                           kind="ExternalInput")
    wrep_d = nc.dram_tensor("wrep", [NPASS * 8, SH], dt.bfloat16,
                            kind="ExternalInput")
    sel_d = nc.dram_tensor("sel", [128, K], dt.float32, kind="ExternalInput")
    selb_d = nc.dram_tensor("selb", [128, K], dt.bfloat16, kind="ExternalInput")
    out_d = nc.dram_tensor("out", [B, 128, NBLK * K], dt.float32,
                           kind="ExternalOutput")

    with tile.TileContext(nc) as tc:
        from contextlib import ExitStack
        with ExitStack() as ctx:
            tp = ctx.enter_context(tc.tile_pool(name="tabs", bufs=2))
            gp = ctx.enter_context(tc.tile_pool(name="gath", bufs=2))
            wp = ctx.enter_context(tc.tile_pool(name="gw", bufs=2))
            fp = ctx.enter_context(tc.tile_pool(name="frac", bufs=2))
            ip = ctx.enter_context(tc.tile_pool(name="idxs", bufs=1))
            sp = ctx.enter_context(tc.tile_pool(name="small", bufs=1))
            pp = ctx.enter_context(tc.tile_pool(name="ps", bufs=1, space="PSUM"))

            # critical-path first: the p=0 table load gates the first gather,
            # so it goes ahead of everything else on the SP DMA queue
            T_tiles = {}
            T_tiles[0] = tp.tile([128, NS], dt.float32, tag="T", name="T0")
            nc.sync.dma_start(T_tiles[0][:, :], tab_d[0, :, :])

            # per-pass index tiles, 3 prefetched up front, rest in-loop
            i16 = {}
            for p in range(min(3, NPASS)):
                i16[p] = ip.tile([128, CW], dt.int16, tag=f"i16_{p}",
                                 name=f"i16_{p}")
                nc.sync.dma_start(i16[p][:, :], i16_d[p, :, :])

            sel_t = sp.tile([128, K], dt.float32, name="sel_t")
            nc.sync.dma_start(sel_t[:, :], sel_d[:, :])
            selb_t = sp.tile([128, K], dt.bfloat16, name="selb_t")
            nc.scalar.dma_start(selb_t[:, :], selb_d[:, :])

            psums = []
            for b in range(B):
                pst = pp.tile([128, NBLK * K], dt.float32, tag=f"ps{b}",
                              name=f"ps{b}")
                psums.append(pst)

            for p in range(NPASS):
                b = p // 8

                # --- (v0, v1) table rows for this pass (host-interleaved) ---
                if p not in T_tiles:
                    T_tiles[p] = tp.tile([128, NS], dt.float32, tag="T",
                                         name=f"T{p}")
                    nc.sync.dma_start(T_tiles[p][:, :], tab_d[p, :, :])
                T = T_tiles[p]
                if p + 3 < NPASS:
                    q = p + 3
                    i16[q] = ip.tile([128, CW], dt.int16, tag=f"i16_{q}",
                                     name=f"i16_{q}")
                    nc.sync.dma_start(i16[q][:, :], i16_d[q, :, :])

                # --- all-lane w tile: 16 strided DMAs (dst partitions j::16
                # <- the 8 compact rows) do the lane expansion. Input staging
                # is charged per exec (~18us/MB/core), so compact beats
                # host-replicated; the extra HWDGE issues hide under the
                # 200us+ gather.
                F = fp.tile([128, SH], dt.bfloat16, tag="F")
                for j in range(16):
                    eng = nc.scalar if j % 2 else nc.sync
                    eng.dma_start(F[j::16, :], wrep_d[p * 8:(p + 1) * 8, :])

                # --- the gather: G[16g+j, q] = T[16g+j, i0[g,q]] ---
                G = gp.tile([128, SH], dt.float32, tag="G")
                nc.gpsimd.ap_gather(
                    G[:, :].rearrange("p (n i) -> p n i", i=1),
                    T[:, :].rearrange("p (n i) -> p n i", i=1),
                    i16[p][:, :],
                    channels=128,
                    num_elems=NS,
                    d=1,
                    num_idxs=SH,
                )

                # --- weighted copy on DVE (separate tile: no PE-order dep).
                # Kept off GPSIMD entirely: a Pool tensor op would force a
                # Q7 ucode-library reload away from the ap_gather library.
                GW = wp.tile([128, SH], dt.bfloat16, tag="GW")
                nc.vector.tensor_mul(GW[:, :], G[:, :], F[:, :])

                # --- PE: raw G vs sel0, then GW vs selpm, accumulated ---
                # NB: start=True resets the WHOLE psum bank, so only the
                # first matmul of each bank's group sets it; only the very
                # last sets stop.
                for blk in range(NBLK):
                    nc.tensor.matmul(
                        psums[b][:, blk * K:(blk + 1) * K],
                        G[:, blk * BLK:(blk + 1) * BLK],
                        sel_t[:, :],
                        start=(p % 8 == 0 and blk == 0),
                        stop=False,
                        skip_group_check=True,
                    )
                for blk in range(NBLK):
                    nc.tensor.matmul(
                        psums[b][:, blk * K:(blk + 1) * K],
                        GW[:, blk * BLK:(blk + 1) * BLK],
                        selb_t[:, :],
                        start=False,
                        stop=(p % 8 == 7 and blk == NBLK - 1),
                        skip_group_check=True,
                    )

                if p % 8 == 7:
                    cp = sp.tile([128, NBLK * K], dt.float32, tag=f"cp{b}",
                                 name=f"cp{b}")
                    nc.scalar.copy(cp[:, :], psums[b][:, :])
                    nc.sync.dma_start(out_d[b, :, :], cp[:, :])

    nc.compile()
    return nc


def _host_prep(rfs, ids, samples_idx):
    rfs = np.asarray(rfs, dtype=np.float32)
    ids = np.asarray(ids).astype(np.int64)
    samples_idx = np.asarray(samples_idx, dtype=np.float32)

    # table rows: tab[p, 16g+k] = rfs[b, k, nc, :]; tab[p, 16g+8+k] = shifted
    s_rows = rfs.transpose(0, 2, 1, 3)                           # b, nc, k, s
    sh_rows = np.zeros_like(s_rows)
    sh_rows[..., : NS - 1] = s_rows[..., 1:]
    both = np.stack([s_rows, sh_rows], axis=2)                   # b, nc, tap, k, s
    tab = np.ascontiguousarray(both.reshape(NPASS, 128, NS))

    idx = samples_idx[ids].reshape(B, NC, NPIX)  # [2, 64, 65536]
    i0 = np.floor(idx)
    i16full = i0.astype(np.int16)
    wfull = (idx - i0).astype(ml_dtypes.bfloat16)

    # slot = 16g + 8t + k;  sel: +1 on t==0 lanes (raw G term)
    # selb: -1 on t==0, +1 on t==1 lanes (w*(v1-v0) term)
    sel = np.zeros((128, K), dtype=np.float32)
    selb = np.zeros((128, K), dtype=np.float32)
    slots = np.arange(128)
    t_of = (slots % 16) // 8
    k_of = slots % 8
    sel[slots, k_of] = (t_of == 0).astype(np.float32)
    selb[slots, k_of] = np.where(t_of == 0, -1.0, 1.0)
    selb = selb.astype(ml_dtypes.bfloat16)

    in_maps = []
    for c in range(NCORES):
        sl16 = i16full[:, :, c * SH:(c + 1) * SH]    # [B, NC, SH]
        # wrapped: [NPASS, 128, CW]: partition 16g+m, free col
        t = sl16.reshape(B, 8, 8, CW, 16)            # b, ncg, g, c, m
        t = t.transpose(0, 1, 2, 4, 3)               # b, ncg, g, m, c
        t = t.reshape(NPASS, 8, 16, CW).reshape(NPASS, 128, CW)
        i16w = np.ascontiguousarray(t)
        # compact frac rows: row p*8+g = w[b, nc, core slice], pixel order
        wrep = np.ascontiguousarray(
            wfull[:, :, c * SH:(c + 1) * SH].reshape(NPASS * 8, SH)
        )
        in_maps.append(dict(tab=tab, i16w=i16w, wrep=wrep, sel=sel, selb=selb))
    return in_maps


def kernel(rfs, ids, samples_idx):
    if "nc" not in _CACHE:
        _CACHE["nc"] = _build_program()
    nc = _CACHE["nc"]

    in_maps = _host_prep(rfs, ids, samples_idx)
    res = run_bass_kernel_spmd(nc, in_maps, core_ids=list(range(NCORES)))

    out = np.empty((B, NPIX, K), dtype=np.float32)
    for c in range(NCORES):
        o = res.results[c]["out"]                     # [B, 128, NBLK*K]
        o = o.reshape(B, 128, NBLK, K).transpose(0, 2, 1, 3)  # b, blk, qlo, k
        out[:, c * SH:(c + 1) * SH, :] = o.reshape(B, SH, K)
    return out.reshape(B, NZ, NX, K)
